# revision 14
# baseline (speedup 1.0000x reference)
"""Trainium2 Bass kernel for nn_GaussianQuantRegularizer_6992206758164.

Eval path of a GaussianQuantRegularizer: reparam sample + per-group
codebook scoring (2 small GEMMs folded into one K=54 fp16-split matmul),
exact argmax over 1024 codes, gather, on 8 NeuronCores data-parallel over
the flattened query dim.

Per core (32768 queries = 512 positions x 64 j-lanes, codebook 1024x4):
  A. contiguous DMA loads of mu/logvar/eps position-blocks [128, 256]
  B. ACT exp + DVE feature math -> query features [a, iv, 1], 3-way fp16
     split (h/m/l) written into per-chunk [128, 54] blocks, PE-transposed
     into K-major lhsT tiles [54, 128]
  C. per chunk (pos-block, j-lane) = 128 queries: PE matmul vs fp16-split
     codebook features [54, 1024] -> scores f32 PSUM [128, 1024];
     DVE max8 + max_index -> exact first-occurrence argmax (uint32)
  D. indirect-DMA gather prior[idx] -> zhat; idx -> indices
"""

import sys
import numpy as np

sys.path.insert(0, "/opt/trn_rl_repo")

import concourse.bass as bass
import concourse.tile as tile
from concourse import mybir
from concourse import bass_utils
from concourse.bass import IndirectOffsetOnAxis

F32 = mybir.dt.float32
F16 = mybir.dt.bfloat16
U32 = mybir.dt.uint32

B, L, C2 = 4, 1024, 512
C = C2 // 2            # 256
G = 4                  # group
CG = C // G            # 64 queries (j-lanes) per position
NS = 1024              # n codebook samples
POS = B * L            # 4096 flattened positions
NCORES = 8
POS_S = POS // NCORES  # 512 positions per core
PB = POS_S // 128      # 4 position blocks per core
NQ = POS_S * CG        # 32768 queries per core
CH = PB * CG           # 256 chunks of 128 queries
LOG2PI = float(np.log(2.0 * np.pi))

# bf16 3-way-split block pairing: score = sum over blocks of q_part*c_part
# (h+m+l)(H+M+L) keeping all products down to ~2^-24 relative.
QPARTS = [0, 0, 1, 0, 1, 2]   # h, h, m, h, m, l
CPARTS = [0, 1, 0, 2, 1, 0]   # H, M, H, L, M, H
NBLK = len(QPARTS)
K = NBLK * 9               # 54


# ---------------------------------------------------------------------------
# walrus workaround: this container's walrus accepts at most ONE semaphore
# wait per instruction. Hoist extras onto same-engine NoOp carriers.
# ---------------------------------------------------------------------------
_fix_counter = [0]


def _fix_sync_waits(nc):
    for f in nc.m.functions:
        for bb in f.blocks:
            insts = list(bb.instructions)
            out = []
            changed = False
            for ins in insts:
                si = ins.sync_info
                waits = list(si.on_wait) if (si and si.on_wait) else []
                if len(waits) > 1:
                    changed = True
                    for w in waits[:-1]:
                        _fix_counter[0] += 1
                        nop = mybir.InstNoOp(name=f"I-waitfix-{_fix_counter[0]}")
                        nop.engine = ins.engine
                        nop.sync_info = mybir.SyncInfo(on_wait=[w], on_update=[])
                        out.append(nop)
                    si.on_wait = waits[-1:]
                out.append(ins)
            if changed:
                try:
                    bb.instructions = out
                except Exception:
                    while len(bb.instructions):
                        bb.instructions.pop()
                    for i in out:
                        bb.instructions.append(i)


_orig_sched = tile.TileContext.schedule_and_allocate


def _sched_patched(self, *a, **k):
    r = _orig_sched(self, *a, **k)
    _fix_sync_waits(self.nc)
    return r


tile.TileContext.schedule_and_allocate = _sched_patched


# ---------------------------------------------------------------------------
# kernel build
# ---------------------------------------------------------------------------

def build_kernel(debug=None):
    nc = bass.Bass()
    tc = tile.TileContext(nc)

    zs = nc.dram_tensor("zs", [POS_S, C2], F32, kind="ExternalInput")
    eps_s = nc.dram_tensor("eps_s", [POS_S, C], F32, kind="ExternalInput")
    prior = nc.dram_tensor("prior", [NS, G], F32, kind="ExternalInput")

    zhat_s = nc.dram_tensor("zhat_s", [POS_S, C], F32, kind="ExternalOutput")
    znq_s = nc.dram_tensor("znq_s", [POS_S, C], F32, kind="ExternalOutput")
    idx_s = nc.dram_tensor("idx_s", [POS_S, CG], U32, kind="ExternalOutput")
    dbg = None
    if debug:
        dbg = nc.dram_tensor("dbg", debug, F32, kind="ExternalOutput")

    with tc:
        _build_body(nc, tc, zs, eps_s, prior, zhat_s, znq_s, idx_s, dbg)
    return nc


def _build_body(nc, tc, zs, eps_s, prior, zhat_s, znq_s, idx_s, dbg):
    from contextlib import ExitStack

    ctx = ExitStack()
    with ctx:
        const_p = ctx.enter_context(tc.tile_pool(name="const", bufs=1))
        feat_p = ctx.enter_context(tc.tile_pool(name="feat", bufs=2))
        ft_p = ctx.enter_context(tc.tile_pool(name="ft", bufs=2))
        work_p = ctx.enter_context(tc.tile_pool(name="work", bufs=2))
        idx_p = ctx.enter_context(tc.tile_pool(name="idx", bufs=64))

        # ============== stage 0: codebook features Frhs [54, 1024] f16 =====
        # f32 feature rows [9, 1024]: 0-3 = S^T, 4-7 = -0.5*(S^2)^T,
        # 8 = 0.5*sum_g S^2 + 2*log(2pi)   (the -beta*nlp_sum term)
        st = const_p.tile([4, NS], F32, tag="st")          # S^T
        nc.sync.dma_start(st[:], prior.rearrange("n g -> g n"))
        s2t = const_p.tile([4, NS], F32, tag="s2t")
        nc.vector.tensor_tensor(s2t[:], st[:], st[:], op=mybir.AluOpType.mult)
        sneg = const_p.tile([4, NS], F32, tag="sneg")
        nc.vector.tensor_scalar_mul(sneg[:], s2t[:], -0.5)
        # nlp row: sum_g S^2 via matmul with ones [4,1]
        ones4 = const_p.tile([4, 1], F32, tag="ones4")
        nc.gpsimd.memset(ones4[:], 1.0)
        row8 = const_p.tile([1, NS], F32, tag="row8")
        with tc.tile_pool(name="ps0pool", bufs=1, space="PSUM") as ps0_p:
            ps0 = ps0_p.tile([1, NS], F32, tag="ps0")
            nc.tensor.matmul(ps0[:, 0:512], ones4[:], s2t[:, 0:512],
                             start=True, stop=True)
            nc.tensor.matmul(ps0[:, 512:NS], ones4[:], s2t[:, 512:NS],
                             start=True, stop=True)
            # row8 = 0.5*ps0 + 2log2pi
            bias2pi = const_p.tile([1, 1], F32, tag="bias2pi")
            nc.gpsimd.memset(bias2pi[:], 2.0 * LOG2PI)
            nc.scalar.activation(row8[:], ps0[:],
                                 mybir.ActivationFunctionType.Copy, scale=0.5)
            nc.vector.tensor_scalar_add(row8[:], row8[:], bias2pi[:])
        # assemble f9 [9, NS] via partition-offset SBUF->SBUF DMAs
        f9 = const_p.tile([9, NS], F32, tag="f9")
        nc.sync.dma_start(f9[0:4, :], st[:])
        nc.sync.dma_start(f9[4:8, :], sneg[:])
        nc.sync.dma_start(f9[8:9, :], row8[:])

        # fp16 split F -> FH/FM/FL [9, 1024] each, then Frhs [54, 1024]
        fh = const_p.tile([9, NS], F16, tag="fh")
        fm = const_p.tile([9, NS], F16, tag="fm")
        fl = const_p.tile([9, NS], F16, tag="fl")
        fr1 = const_p.tile([9, NS], F32, tag="fr1")
        fr2 = const_p.tile([9, NS], F32, tag="fr2")
        nc.vector.tensor_copy(fh[:], f9[:])
        nc.vector.tensor_tensor(fr1[:], f9[:], fh[:], op=mybir.AluOpType.subtract)
        nc.vector.tensor_copy(fm[:], fr1[:])
        nc.vector.tensor_tensor(fr2[:], fr1[:], fm[:], op=mybir.AluOpType.subtract)
        nc.vector.tensor_copy(fl[:], fr2[:])
        frhs = const_p.tile([K, NS], F16, tag="frhs")
        cparts = [fh, fm, fl]
        for bi in range(NBLK):
            nc.sync.dma_start(frhs[9 * bi:9 * (bi + 1), :], cparts[CPARTS[bi]][:])

        # identity [128, 128] f16 for PE transpose
        ident = const_p.tile([128, 128], F16, tag="ident")
        onesb = const_p.tile([128, 128], F16, tag="onesb")
        nc.gpsimd.memset(onesb[:], 1.0)
        nc.gpsimd.affine_select(ident[:], onesb[:], pattern=[[1, 128]],
                                compare_op=mybir.AluOpType.is_equal, fill=0.0,
                                base=0, channel_multiplier=-1)

        # collected argmax indices: chunk (pb, j) -> cols 8*(pb*64+j)..+8
        idxcoll = const_p.tile([128, CH * 8], U32, tag="idxcoll")

        psum_p = ctx.enter_context(tc.tile_pool(name="psum", bufs=3, space="PSUM"))
        tpsum_p = ctx.enter_context(tc.tile_pool(name="tpsum", bufs=2, space="PSUM"))

        # ============== per position block: features -> scores -> argmax ====
        for pb in range(PB):
            rows = slice(128 * pb, 128 * (pb + 1))
            mu = feat_p.tile([128, C], F32, tag="mu")
            lv = feat_p.tile([128, C], F32, tag="lv")
            ep = feat_p.tile([128, C], F32, tag="ep")
            nc.sync.dma_start(mu[:], zs[rows, 0:C])
            nc.sync.dma_start(lv[:], zs[rows, C:C2])
            nc.sync.dma_start(ep[:], eps_s[rows, :])

            iv = feat_p.tile([128, C], F32, tag="iv")
            nc.scalar.activation(iv[:], lv[:], mybir.ActivationFunctionType.Exp,
                                 scale=-1.0)
            av = feat_p.tile([128, C], F32, tag="av")
            nc.vector.tensor_tensor(av[:], mu[:], iv[:], op=mybir.AluOpType.mult)

            std = feat_p.tile([128, C], F32, tag="std")
            nc.scalar.activation(std[:], lv[:], mybir.ActivationFunctionType.Exp,
                                 scale=0.5)
            znq = feat_p.tile([128, C], F32, tag="znq")
            nc.vector.tensor_tensor(znq[:], ep[:], std[:], op=mybir.AluOpType.mult)
            nc.vector.tensor_tensor(znq[:], znq[:], mu[:], op=mybir.AluOpType.add)
            nc.sync.dma_start(znq_s[rows, :], znq[:])

            # --- 3-way fp16 split into phi [128, 64(j), 54] ---
            phi = feat_p.tile([128, CG, K], F16, tag="phi")
            for bi in range(NBLK):
                nc.vector.memset(phi[:, :, 9 * bi + 8],
                                 1.0 if QPARTS[bi] == 0 else 0.0)

            def split_feat(src, featoff):
                # src [128, 256] viewed (g, j) -> slot cols featoff..featoff+4
                srcv = src[:].rearrange("p (g j) -> p j g", g=G)
                parts = [[bi for bi in range(NBLK) if QPARTS[bi] == w]
                         for w in range(3)]
                dsts = [[phi[:, :, 9 * bi + featoff:9 * bi + featoff + 4]
                         for bi in blks] for blks in parts]
                r1 = feat_p.tile([128, CG, G], F32, tag="spl_r1")
                r2 = feat_p.tile([128, CG, G], F32, tag="spl_r2")
                nc.vector.tensor_copy(dsts[0][0], srcv)
                for d in dsts[0][1:]:
                    nc.vector.tensor_copy(d, dsts[0][0])
                nc.vector.tensor_tensor(r1[:], srcv, dsts[0][0],
                                        op=mybir.AluOpType.subtract)
                nc.vector.tensor_copy(dsts[1][0], r1[:])
                for d in dsts[1][1:]:
                    nc.vector.tensor_copy(d, dsts[1][0])
                nc.vector.tensor_tensor(r2[:], r1[:], dsts[1][0],
                                        op=mybir.AluOpType.subtract)
                nc.vector.tensor_copy(dsts[2][0], r2[:])

            split_feat(av, 0)
            split_feat(iv, 4)

            # --- transpose each chunk's [128, 54] -> ft [54, 64*128] ---
            ft = ft_p.tile([K, CG * 128], F16, tag="ft")
            for j in range(CG):
                tp = tpsum_p.tile([K, 128], F16, tag="tp")
                nc.tensor.transpose(tp[:], phi[:, j, :], ident[:])
                nc.scalar.copy(ft[:, 128 * j:128 * (j + 1)], tp[:])

            # --- scores + argmax + gather per chunk ---
            rows = slice(128 * pb, 128 * (pb + 1))
            zq = work_p.tile([128, C], F32, tag="zq")
            zqv = zq[:].rearrange("p (j g) -> p j g", g=G)
            for j in range(CG):
                ps = psum_p.tile([128, NS], F32, tag="ps")
                lhsT = ft[:, 128 * j:128 * (j + 1)]
                nc.tensor.matmul(ps[:, 0:512], lhsT, frhs[:, 0:512],
                                 start=True, stop=True)
                nc.tensor.matmul(ps[:, 512:NS], lhsT, frhs[:, 512:NS],
                                 start=True, stop=True)
                m8 = idx_p.tile([128, 8], F32, tag="m8")
                nc.vector.max(m8[:], ps[:])
                cid = pb * CG + j
                i8 = idxcoll[:, 8 * cid:8 * cid + 8]
                nc.vector.max_index(i8, m8[:], ps[:])
                nc.gpsimd.indirect_dma_start(
                    out=zqv[:, j, :],
                    out_offset=None,
                    in_=prior[:],
                    in_offset=IndirectOffsetOnAxis(ap=idxcoll[:, 8 * cid:8 * cid + 1],
                                                   axis=0),
                )
                if dbg is not None and pb == 0 and j == 0:
                    nc.sync.dma_start(dbg[0:128, 0:NS], ps[:])
            zsb = work_p.tile([128, C], F32, tag="zsb")
            nc.vector.tensor_copy(zsb[:].rearrange("p (g j) -> p j g", g=G),
                                  zq[:].rearrange("p (j g) -> p j g", g=G))
            nc.sync.dma_start(zhat_s[rows, :], zsb[:])
            idxsm = idx_p.tile([128, CG], U32, tag="idxsm")
            nc.vector.tensor_copy(
                idxsm[:],
                idxcoll[:].rearrange("p (c e) -> p c e", e=8)[:, CG * pb:CG * (pb + 1), 0])
            nc.sync.dma_start(idx_s[rows, :], idxsm[:])

# ---------------------------------------------------------------------------
# host wrapper
# ---------------------------------------------------------------------------
_nc_cache = {}


def _get_nc(debug=None):
    key = ("k", tuple(debug) if debug else None)
    if key not in _nc_cache:
        _nc_cache[key] = build_kernel(debug)
    return _nc_cache[key]


def kernel(z, eps, prior_samples, _trace=False, _debug=None):
    z = np.ascontiguousarray(np.asarray(z, dtype=np.float32))
    eps = np.ascontiguousarray(np.asarray(eps, dtype=np.float32))
    prior = np.ascontiguousarray(np.asarray(prior_samples, dtype=np.float32))

    zflat = z.reshape(POS, C2)
    eflat = eps.reshape(POS, C)

    nc = _get_nc(_debug)
    in_maps = []
    for m in range(NCORES):
        sl = slice(POS_S * m, POS_S * (m + 1))
        in_maps.append({
            "zs": np.ascontiguousarray(zflat[sl]),
            "eps_s": np.ascontiguousarray(eflat[sl]),
            "prior": prior,
        })
    res = bass_utils.run_bass_kernel_spmd(
        nc, in_maps, core_ids=list(range(NCORES)), trace=_trace)

    zhat = np.concatenate([res.results[m]["zhat_s"] for m in range(NCORES)], axis=0)
    znq = np.concatenate([res.results[m]["znq_s"] for m in range(NCORES)], axis=0)
    idx = np.concatenate([res.results[m]["idx_s"] for m in range(NCORES)], axis=0)

    zhat = zhat.reshape(B, L, C)
    znq = znq.reshape(B, L, C)
    indices = idx.view(np.int32).reshape(B, L, CG)
    out = (zhat, znq, indices)
    if _trace or _debug:
        return out, res
    return out


# revision 15
# speedup vs baseline: 1.0087x; 1.0087x over previous
"""Trainium2 Bass kernel for nn_GaussianQuantRegularizer_6992206758164.

Eval path of a GaussianQuantRegularizer: reparam sample + per-group
codebook scoring (2 small GEMMs folded into one K=54 fp16-split matmul),
exact argmax over 1024 codes, gather, on 8 NeuronCores data-parallel over
the flattened query dim.

Per core (32768 queries = 512 positions x 64 j-lanes, codebook 1024x4):
  A. contiguous DMA loads of mu/logvar/eps position-blocks [128, 256]
  B. ACT exp + DVE feature math -> query features [a, iv, 1], 3-way fp16
     split (h/m/l) written into per-chunk [128, 54] blocks, PE-transposed
     into K-major lhsT tiles [54, 128]
  C. per chunk (pos-block, j-lane) = 128 queries: PE matmul vs fp16-split
     codebook features [54, 1024] -> scores f32 PSUM [128, 1024];
     DVE max8 + max_index -> exact first-occurrence argmax (uint32)
  D. indirect-DMA gather prior[idx] -> zhat; idx -> indices
"""

import sys
import numpy as np

sys.path.insert(0, "/opt/trn_rl_repo")

import concourse.bass as bass
import concourse.tile as tile
from concourse import mybir
from concourse import bass_utils
from concourse.bass import IndirectOffsetOnAxis

F32 = mybir.dt.float32
F16 = mybir.dt.bfloat16
U32 = mybir.dt.uint32

B, L, C2 = 4, 1024, 512
C = C2 // 2            # 256
G = 4                  # group
CG = C // G            # 64 queries (j-lanes) per position
NS = 1024              # n codebook samples
POS = B * L            # 4096 flattened positions
NCORES = 8
POS_S = POS // NCORES  # 512 positions per core
PB = POS_S // 128      # 4 position blocks per core
NQ = POS_S * CG        # 32768 queries per core
CH = PB * CG           # 256 chunks of 128 queries
LOG2PI = float(np.log(2.0 * np.pi))

# bf16 3-way-split block pairing: score = sum over blocks of q_part*c_part
# (h+m+l)(H+M+L) keeping all products down to ~2^-24 relative.
QPARTS = [0, 0, 1, 0, 1, 2]   # h, h, m, h, m, l
CPARTS = [0, 1, 0, 2, 1, 0]   # H, M, H, L, M, H
NBLK = len(QPARTS)
K = NBLK * 9               # 54


# ---------------------------------------------------------------------------
# walrus workaround: this container's walrus accepts at most ONE semaphore
# wait per instruction. Hoist extras onto same-engine NoOp carriers.
# ---------------------------------------------------------------------------
_fix_counter = [0]


def _fix_sync_waits(nc):
    for f in nc.m.functions:
        for bb in f.blocks:
            insts = list(bb.instructions)
            out = []
            changed = False
            for ins in insts:
                si = ins.sync_info
                waits = list(si.on_wait) if (si and si.on_wait) else []
                if len(waits) > 1:
                    changed = True
                    for w in waits[:-1]:
                        _fix_counter[0] += 1
                        nop = mybir.InstNoOp(name=f"I-waitfix-{_fix_counter[0]}")
                        nop.engine = ins.engine
                        nop.sync_info = mybir.SyncInfo(on_wait=[w], on_update=[])
                        out.append(nop)
                    si.on_wait = waits[-1:]
                out.append(ins)
            if changed:
                try:
                    bb.instructions = out
                except Exception:
                    while len(bb.instructions):
                        bb.instructions.pop()
                    for i in out:
                        bb.instructions.append(i)


_orig_sched = tile.TileContext.schedule_and_allocate


def _sched_patched(self, *a, **k):
    r = _orig_sched(self, *a, **k)
    _fix_sync_waits(self.nc)
    return r


tile.TileContext.schedule_and_allocate = _sched_patched


# ---------------------------------------------------------------------------
# kernel build
# ---------------------------------------------------------------------------

def build_kernel(debug=None):
    nc = bass.Bass()
    tc = tile.TileContext(nc)

    zs = nc.dram_tensor("zs", [POS_S, C2], F32, kind="ExternalInput")
    eps_s = nc.dram_tensor("eps_s", [POS_S, C], F32, kind="ExternalInput")
    prior = nc.dram_tensor("prior", [NS, G], F32, kind="ExternalInput")

    zhat_s = nc.dram_tensor("zhat_s", [POS_S, C], F32, kind="ExternalOutput")
    znq_s = nc.dram_tensor("znq_s", [POS_S, C], F32, kind="ExternalOutput")
    idx_s = nc.dram_tensor("idx_s", [POS_S, CG], U32, kind="ExternalOutput")
    dbg = None
    if debug:
        dbg = nc.dram_tensor("dbg", debug, F32, kind="ExternalOutput")

    with tc:
        _build_body(nc, tc, zs, eps_s, prior, zhat_s, znq_s, idx_s, dbg)
    return nc


def _build_body(nc, tc, zs, eps_s, prior, zhat_s, znq_s, idx_s, dbg):
    from contextlib import ExitStack

    ctx = ExitStack()
    with ctx:
        const_p = ctx.enter_context(tc.tile_pool(name="const", bufs=1))
        feat_p = ctx.enter_context(tc.tile_pool(name="feat", bufs=2))
        ft_p = ctx.enter_context(tc.tile_pool(name="ft", bufs=2))
        work_p = ctx.enter_context(tc.tile_pool(name="work", bufs=2))
        idx_p = ctx.enter_context(tc.tile_pool(name="idx", bufs=64))

        # ============== stage 0: codebook features Frhs [54, 1024] f16 =====
        # f32 feature rows [9, 1024]: 0-3 = S^T, 4-7 = -0.5*(S^2)^T,
        # 8 = 0.5*sum_g S^2 + 2*log(2pi)   (the -beta*nlp_sum term)
        st = const_p.tile([4, NS], F32, tag="st")          # S^T
        nc.sync.dma_start(st[:], prior.rearrange("n g -> g n"))
        s2t = const_p.tile([4, NS], F32, tag="s2t")
        nc.vector.tensor_tensor(s2t[:], st[:], st[:], op=mybir.AluOpType.mult)
        sneg = const_p.tile([4, NS], F32, tag="sneg")
        nc.vector.tensor_scalar_mul(sneg[:], s2t[:], -0.5)
        # nlp row: sum_g S^2 via matmul with ones [4,1]
        ones4 = const_p.tile([4, 1], F32, tag="ones4")
        nc.gpsimd.memset(ones4[:], 1.0)
        row8 = const_p.tile([1, NS], F32, tag="row8")
        with tc.tile_pool(name="ps0pool", bufs=1, space="PSUM") as ps0_p:
            ps0 = ps0_p.tile([1, NS], F32, tag="ps0")
            nc.tensor.matmul(ps0[:, 0:512], ones4[:], s2t[:, 0:512],
                             start=True, stop=True)
            nc.tensor.matmul(ps0[:, 512:NS], ones4[:], s2t[:, 512:NS],
                             start=True, stop=True)
            # row8 = 0.5*ps0 + 2log2pi
            bias2pi = const_p.tile([1, 1], F32, tag="bias2pi")
            nc.gpsimd.memset(bias2pi[:], 2.0 * LOG2PI)
            nc.scalar.activation(row8[:], ps0[:],
                                 mybir.ActivationFunctionType.Copy, scale=0.5)
            nc.vector.tensor_scalar_add(row8[:], row8[:], bias2pi[:])
        # assemble f9 [9, NS] via partition-offset SBUF->SBUF DMAs
        f9 = const_p.tile([9, NS], F32, tag="f9")
        nc.sync.dma_start(f9[0:4, :], st[:])
        nc.sync.dma_start(f9[4:8, :], sneg[:])
        nc.sync.dma_start(f9[8:9, :], row8[:])

        # fp16 split F -> FH/FM/FL [9, 1024] each, then Frhs [54, 1024]
        fh = const_p.tile([9, NS], F16, tag="fh")
        fm = const_p.tile([9, NS], F16, tag="fm")
        fl = const_p.tile([9, NS], F16, tag="fl")
        fr1 = const_p.tile([9, NS], F32, tag="fr1")
        fr2 = const_p.tile([9, NS], F32, tag="fr2")
        nc.vector.tensor_copy(fh[:], f9[:])
        nc.vector.tensor_tensor(fr1[:], f9[:], fh[:], op=mybir.AluOpType.subtract)
        nc.vector.tensor_copy(fm[:], fr1[:])
        nc.vector.tensor_tensor(fr2[:], fr1[:], fm[:], op=mybir.AluOpType.subtract)
        nc.vector.tensor_copy(fl[:], fr2[:])
        frhs = const_p.tile([K, NS], F16, tag="frhs")
        cparts = [fh, fm, fl]
        for bi in range(NBLK):
            nc.sync.dma_start(frhs[9 * bi:9 * (bi + 1), :], cparts[CPARTS[bi]][:])

        # identity [128, 128] f16 for PE transpose
        ident = const_p.tile([128, 128], F16, tag="ident")
        onesb = const_p.tile([128, 128], F16, tag="onesb")
        nc.gpsimd.memset(onesb[:], 1.0)
        nc.gpsimd.affine_select(ident[:], onesb[:], pattern=[[1, 128]],
                                compare_op=mybir.AluOpType.is_equal, fill=0.0,
                                base=0, channel_multiplier=-1)

        # collected argmax indices: chunk (pb, j) -> cols 8*(pb*64+j)..+8
        idxcoll = const_p.tile([128, CH * 8], U32, tag="idxcoll")

        psum_p = ctx.enter_context(tc.tile_pool(name="psum", bufs=3, space="PSUM"))
        tpsum_p = ctx.enter_context(tc.tile_pool(name="tpsum", bufs=2, space="PSUM"))

        # ============== per position block: features -> scores -> argmax ====
        for pb in range(PB):
            rows = slice(128 * pb, 128 * (pb + 1))
            mu = feat_p.tile([128, C], F32, tag="mu")
            lv = feat_p.tile([128, C], F32, tag="lv")
            ep = feat_p.tile([128, C], F32, tag="ep")
            nc.sync.dma_start(mu[:], zs[rows, 0:C])
            nc.sync.dma_start(lv[:], zs[rows, C:C2])
            nc.sync.dma_start(ep[:], eps_s[rows, :])

            iv = feat_p.tile([128, C], F32, tag="iv")
            nc.scalar.activation(iv[:], lv[:], mybir.ActivationFunctionType.Exp,
                                 scale=-1.0)
            av = feat_p.tile([128, C], F32, tag="av")
            nc.vector.tensor_tensor(av[:], mu[:], iv[:], op=mybir.AluOpType.mult)

            std = feat_p.tile([128, C], F32, tag="std")
            nc.scalar.activation(std[:], lv[:], mybir.ActivationFunctionType.Exp,
                                 scale=0.5)
            znq = feat_p.tile([128, C], F32, tag="znq")
            nc.vector.tensor_tensor(znq[:], ep[:], std[:], op=mybir.AluOpType.mult)
            nc.vector.tensor_tensor(znq[:], znq[:], mu[:], op=mybir.AluOpType.add)
            nc.sync.dma_start(znq_s[rows, :], znq[:])

            # --- 3-way fp16 split into phi [128, 64(j), 54] ---
            phi = feat_p.tile([128, CG, K], F16, tag="phi")
            for bi in range(NBLK):
                nc.vector.memset(phi[:, :, 9 * bi + 8],
                                 1.0 if QPARTS[bi] == 0 else 0.0)

            def split_feat(src, featoff):
                # src [128, 256] viewed (g, j) -> slot cols featoff..featoff+4
                srcv = src[:].rearrange("p (g j) -> p j g", g=G)
                parts = [[bi for bi in range(NBLK) if QPARTS[bi] == w]
                         for w in range(3)]
                dsts = [[phi[:, :, 9 * bi + featoff:9 * bi + featoff + 4]
                         for bi in blks] for blks in parts]
                r1 = feat_p.tile([128, CG, G], F32, tag="spl_r1")
                r2 = feat_p.tile([128, CG, G], F32, tag="spl_r2")
                nc.vector.tensor_copy(dsts[0][0], srcv)
                for d in dsts[0][1:]:
                    nc.vector.tensor_copy(d, dsts[0][0])
                nc.vector.tensor_tensor(r1[:], srcv, dsts[0][0],
                                        op=mybir.AluOpType.subtract)
                nc.vector.tensor_copy(dsts[1][0], r1[:])
                for d in dsts[1][1:]:
                    nc.vector.tensor_copy(d, dsts[1][0])
                nc.vector.tensor_tensor(r2[:], r1[:], dsts[1][0],
                                        op=mybir.AluOpType.subtract)
                nc.vector.tensor_copy(dsts[2][0], r2[:])

            split_feat(av, 0)
            split_feat(iv, 4)

            # --- per chunk: transpose (SW-pipelined 1 ahead) + scores +
            #     argmax + gather ---
            rows = slice(128 * pb, 128 * (pb + 1))
            zq = work_p.tile([128, C], F32, tag="zq")
            zqv = zq[:].rearrange("p (j g) -> p j g", g=G)
            ft = ft_p.tile([K, CG * 128], F16, tag="ft")
            tp = tpsum_p.tile([K, 128], F16, tag="tp")
            nc.tensor.transpose(tp[:], phi[:, 0, :], ident[:])
            nc.scalar.copy(ft[:, 0:128], tp[:])
            for j in range(CG):
                if j + 1 < CG:
                    tp = tpsum_p.tile([K, 128], F16, tag="tp")
                    nc.tensor.transpose(tp[:], phi[:, j + 1, :], ident[:])
                    nc.scalar.copy(ft[:, 128 * (j + 1):128 * (j + 2)], tp[:])
                ps = psum_p.tile([128, NS], F32, tag="ps")
                lhsT = ft[:, 128 * j:128 * (j + 1)]
                nc.tensor.matmul(ps[:, 0:512], lhsT, frhs[:, 0:512],
                                 start=True, stop=True)
                nc.tensor.matmul(ps[:, 512:NS], lhsT, frhs[:, 512:NS],
                                 start=True, stop=True)
                m8 = idx_p.tile([128, 8], F32, tag="m8")
                nc.vector.max(m8[:], ps[:])
                cid = pb * CG + j
                i8 = idxcoll[:, 8 * cid:8 * cid + 8]
                nc.vector.max_index(i8, m8[:], ps[:])
                nc.gpsimd.indirect_dma_start(
                    out=zqv[:, j, :],
                    out_offset=None,
                    in_=prior[:],
                    in_offset=IndirectOffsetOnAxis(ap=idxcoll[:, 8 * cid:8 * cid + 1],
                                                   axis=0),
                )
                if dbg is not None and pb == 0 and j == 0:
                    nc.sync.dma_start(dbg[0:128, 0:NS], ps[:])
            zsb = work_p.tile([128, C], F32, tag="zsb")
            nc.scalar.copy(zsb[:].rearrange("p (g j) -> p j g", g=G),
                           zq[:].rearrange("p (j g) -> p j g", g=G))
            nc.sync.dma_start(zhat_s[rows, :], zsb[:])
            idxsm = idx_p.tile([128, CG], U32, tag="idxsm")
            nc.vector.tensor_copy(
                idxsm[:],
                idxcoll[:].rearrange("p (c e) -> p c e", e=8)[:, CG * pb:CG * (pb + 1), 0])
            nc.sync.dma_start(idx_s[rows, :], idxsm[:])

# ---------------------------------------------------------------------------
# host wrapper
# ---------------------------------------------------------------------------
_nc_cache = {}


def _get_nc(debug=None):
    key = ("k", tuple(debug) if debug else None)
    if key not in _nc_cache:
        _nc_cache[key] = build_kernel(debug)
    return _nc_cache[key]


def kernel(z, eps, prior_samples, _trace=False, _debug=None):
    z = np.ascontiguousarray(np.asarray(z, dtype=np.float32))
    eps = np.ascontiguousarray(np.asarray(eps, dtype=np.float32))
    prior = np.ascontiguousarray(np.asarray(prior_samples, dtype=np.float32))

    zflat = z.reshape(POS, C2)
    eflat = eps.reshape(POS, C)

    nc = _get_nc(_debug)
    in_maps = []
    for m in range(NCORES):
        sl = slice(POS_S * m, POS_S * (m + 1))
        in_maps.append({
            "zs": np.ascontiguousarray(zflat[sl]),
            "eps_s": np.ascontiguousarray(eflat[sl]),
            "prior": prior,
        })
    res = bass_utils.run_bass_kernel_spmd(
        nc, in_maps, core_ids=list(range(NCORES)), trace=_trace)

    zhat = np.concatenate([res.results[m]["zhat_s"] for m in range(NCORES)], axis=0)
    znq = np.concatenate([res.results[m]["znq_s"] for m in range(NCORES)], axis=0)
    idx = np.concatenate([res.results[m]["idx_s"] for m in range(NCORES)], axis=0)

    zhat = zhat.reshape(B, L, C)
    znq = znq.reshape(B, L, C)
    indices = idx.view(np.int32).reshape(B, L, CG)
    out = (zhat, znq, indices)
    if _trace or _debug:
        return out, res
    return out


# revision 16
# speedup vs baseline: 1.0118x; 1.0031x over previous
"""Trainium2 Bass kernel for nn_GaussianQuantRegularizer_6992206758164.

Eval path of a GaussianQuantRegularizer: reparam sample + per-group
codebook scoring (2 small GEMMs folded into one K=54 fp16-split matmul),
exact argmax over 1024 codes, gather, on 8 NeuronCores data-parallel over
the flattened query dim.

Per core (32768 queries = 512 positions x 64 j-lanes, codebook 1024x4):
  A. contiguous DMA loads of mu/logvar/eps position-blocks [128, 256]
  B. ACT exp + DVE feature math -> query features [a, iv, 1], 3-way fp16
     split (h/m/l) written into per-chunk [128, 54] blocks, PE-transposed
     into K-major lhsT tiles [54, 128]
  C. per chunk (pos-block, j-lane) = 128 queries: PE matmul vs fp16-split
     codebook features [54, 1024] -> scores f32 PSUM [128, 1024];
     DVE max8 + max_index -> exact first-occurrence argmax (uint32)
  D. indirect-DMA gather prior[idx] -> zhat; idx -> indices
"""

import sys
import numpy as np

sys.path.insert(0, "/opt/trn_rl_repo")

import concourse.bass as bass
import concourse.tile as tile
from concourse import mybir
from concourse import bass_utils
from concourse.bass import IndirectOffsetOnAxis

F32 = mybir.dt.float32
F16 = mybir.dt.bfloat16
U32 = mybir.dt.uint32

B, L, C2 = 4, 1024, 512
C = C2 // 2            # 256
G = 4                  # group
CG = C // G            # 64 queries (j-lanes) per position
NS = 1024              # n codebook samples
POS = B * L            # 4096 flattened positions
NCORES = 8
POS_S = POS // NCORES  # 512 positions per core
PB = POS_S // 128      # 4 position blocks per core
NQ = POS_S * CG        # 32768 queries per core
CH = PB * CG           # 256 chunks of 128 queries
LOG2PI = float(np.log(2.0 * np.pi))

# bf16 3-way-split block pairing: score = sum over blocks of q_part*c_part
# (h+m+l)(H+M+L) keeping all products down to ~2^-24 relative.
QPARTS = [0, 0, 1, 0, 1, 2]   # h, h, m, h, m, l
CPARTS = [0, 1, 0, 2, 1, 0]   # H, M, H, L, M, H
NBLK = len(QPARTS)
K = NBLK * 9               # 54


# ---------------------------------------------------------------------------
# walrus workaround: this container's walrus accepts at most ONE semaphore
# wait per instruction. Hoist extras onto same-engine NoOp carriers.
# ---------------------------------------------------------------------------
_fix_counter = [0]


def _fix_sync_waits(nc):
    for f in nc.m.functions:
        for bb in f.blocks:
            insts = list(bb.instructions)
            out = []
            changed = False
            for ins in insts:
                si = ins.sync_info
                waits = list(si.on_wait) if (si and si.on_wait) else []
                if len(waits) > 1:
                    changed = True
                    for w in waits[:-1]:
                        _fix_counter[0] += 1
                        nop = mybir.InstNoOp(name=f"I-waitfix-{_fix_counter[0]}")
                        nop.engine = ins.engine
                        nop.sync_info = mybir.SyncInfo(on_wait=[w], on_update=[])
                        out.append(nop)
                    si.on_wait = waits[-1:]
                out.append(ins)
            if changed:
                try:
                    bb.instructions = out
                except Exception:
                    while len(bb.instructions):
                        bb.instructions.pop()
                    for i in out:
                        bb.instructions.append(i)


_orig_sched = tile.TileContext.schedule_and_allocate


def _sched_patched(self, *a, **k):
    r = _orig_sched(self, *a, **k)
    _fix_sync_waits(self.nc)
    return r


tile.TileContext.schedule_and_allocate = _sched_patched


# ---------------------------------------------------------------------------
# kernel build
# ---------------------------------------------------------------------------

def build_kernel(debug=None):
    nc = bass.Bass()
    tc = tile.TileContext(nc)

    zs = nc.dram_tensor("zs", [POS_S, C2], F32, kind="ExternalInput")
    eps_s = nc.dram_tensor("eps_s", [POS_S, C], F32, kind="ExternalInput")
    prior = nc.dram_tensor("prior", [NS, G], F32, kind="ExternalInput")

    zhat_s = nc.dram_tensor("zhat_s", [POS_S, C], F32, kind="ExternalOutput")
    znq_s = nc.dram_tensor("znq_s", [POS_S, C], F32, kind="ExternalOutput")
    idx_s = nc.dram_tensor("idx_s", [POS_S, CG], U32, kind="ExternalOutput")
    dbg = None
    if debug:
        dbg = nc.dram_tensor("dbg", debug, F32, kind="ExternalOutput")

    with tc:
        _build_body(nc, tc, zs, eps_s, prior, zhat_s, znq_s, idx_s, dbg)
    return nc


def _build_body(nc, tc, zs, eps_s, prior, zhat_s, znq_s, idx_s, dbg):
    from contextlib import ExitStack

    ctx = ExitStack()
    with ctx:
        const_p = ctx.enter_context(tc.tile_pool(name="const", bufs=1))
        feat_p = ctx.enter_context(tc.tile_pool(name="feat", bufs=3))
        ft_p = ctx.enter_context(tc.tile_pool(name="ft", bufs=3))
        work_p = ctx.enter_context(tc.tile_pool(name="work", bufs=3))
        idx_p = ctx.enter_context(tc.tile_pool(name="idx", bufs=64))

        # ============== stage 0: codebook features Frhs [54, 1024] f16 =====
        # f32 feature rows [9, 1024]: 0-3 = S^T, 4-7 = -0.5*(S^2)^T,
        # 8 = 0.5*sum_g S^2 + 2*log(2pi)   (the -beta*nlp_sum term)
        st = const_p.tile([4, NS], F32, tag="st")          # S^T
        nc.sync.dma_start(st[:], prior.rearrange("n g -> g n"))
        s2t = const_p.tile([4, NS], F32, tag="s2t")
        nc.vector.tensor_tensor(s2t[:], st[:], st[:], op=mybir.AluOpType.mult)
        sneg = const_p.tile([4, NS], F32, tag="sneg")
        nc.vector.tensor_scalar_mul(sneg[:], s2t[:], -0.5)
        # nlp row: sum_g S^2 via matmul with ones [4,1]
        ones4 = const_p.tile([4, 1], F32, tag="ones4")
        nc.gpsimd.memset(ones4[:], 1.0)
        row8 = const_p.tile([1, NS], F32, tag="row8")
        with tc.tile_pool(name="ps0pool", bufs=1, space="PSUM") as ps0_p:
            ps0 = ps0_p.tile([1, NS], F32, tag="ps0")
            nc.tensor.matmul(ps0[:, 0:512], ones4[:], s2t[:, 0:512],
                             start=True, stop=True)
            nc.tensor.matmul(ps0[:, 512:NS], ones4[:], s2t[:, 512:NS],
                             start=True, stop=True)
            # row8 = 0.5*ps0 + 2log2pi
            bias2pi = const_p.tile([1, 1], F32, tag="bias2pi")
            nc.gpsimd.memset(bias2pi[:], 2.0 * LOG2PI)
            nc.scalar.activation(row8[:], ps0[:],
                                 mybir.ActivationFunctionType.Copy, scale=0.5)
            nc.vector.tensor_scalar_add(row8[:], row8[:], bias2pi[:])
        # assemble f9 [9, NS] via partition-offset SBUF->SBUF DMAs
        f9 = const_p.tile([9, NS], F32, tag="f9")
        nc.sync.dma_start(f9[0:4, :], st[:])
        nc.sync.dma_start(f9[4:8, :], sneg[:])
        nc.sync.dma_start(f9[8:9, :], row8[:])

        # fp16 split F -> FH/FM/FL [9, 1024] each, then Frhs [54, 1024]
        fh = const_p.tile([9, NS], F16, tag="fh")
        fm = const_p.tile([9, NS], F16, tag="fm")
        fl = const_p.tile([9, NS], F16, tag="fl")
        fr1 = const_p.tile([9, NS], F32, tag="fr1")
        fr2 = const_p.tile([9, NS], F32, tag="fr2")
        nc.vector.tensor_copy(fh[:], f9[:])
        nc.vector.tensor_tensor(fr1[:], f9[:], fh[:], op=mybir.AluOpType.subtract)
        nc.vector.tensor_copy(fm[:], fr1[:])
        nc.vector.tensor_tensor(fr2[:], fr1[:], fm[:], op=mybir.AluOpType.subtract)
        nc.vector.tensor_copy(fl[:], fr2[:])
        frhs = const_p.tile([K, NS], F16, tag="frhs")
        cparts = [fh, fm, fl]
        for bi in range(NBLK):
            nc.sync.dma_start(frhs[9 * bi:9 * (bi + 1), :], cparts[CPARTS[bi]][:])

        # identity [128, 128] f16 for PE transpose
        ident = const_p.tile([128, 128], F16, tag="ident")
        onesb = const_p.tile([128, 128], F16, tag="onesb")
        nc.gpsimd.memset(onesb[:], 1.0)
        nc.gpsimd.affine_select(ident[:], onesb[:], pattern=[[1, 128]],
                                compare_op=mybir.AluOpType.is_equal, fill=0.0,
                                base=0, channel_multiplier=-1)

        # collected argmax indices: chunk (pb, j) -> cols 8*(pb*64+j)..+8
        idxcoll = const_p.tile([128, CH * 8], U32, tag="idxcoll")

        psum_p = ctx.enter_context(tc.tile_pool(name="psum", bufs=3, space="PSUM"))
        tpsum_p = ctx.enter_context(tc.tile_pool(name="tpsum", bufs=2, space="PSUM"))

        # ============== per position block: features -> scores -> argmax ====
        for pb in range(PB):
            rows = slice(128 * pb, 128 * (pb + 1))
            mu = feat_p.tile([128, C], F32, tag="mu")
            lv = feat_p.tile([128, C], F32, tag="lv")
            ep = feat_p.tile([128, C], F32, tag="ep")
            nc.sync.dma_start(mu[:], zs[rows, 0:C])
            nc.sync.dma_start(lv[:], zs[rows, C:C2])
            nc.sync.dma_start(ep[:], eps_s[rows, :])

            iv = feat_p.tile([128, C], F32, tag="iv")
            nc.scalar.activation(iv[:], lv[:], mybir.ActivationFunctionType.Exp,
                                 scale=-1.0)
            av = feat_p.tile([128, C], F32, tag="av")
            nc.vector.tensor_tensor(av[:], mu[:], iv[:], op=mybir.AluOpType.mult)

            std = feat_p.tile([128, C], F32, tag="std")
            nc.scalar.activation(std[:], lv[:], mybir.ActivationFunctionType.Exp,
                                 scale=0.5)
            znq = feat_p.tile([128, C], F32, tag="znq")
            nc.vector.tensor_tensor(znq[:], ep[:], std[:], op=mybir.AluOpType.mult)
            nc.vector.tensor_tensor(znq[:], znq[:], mu[:], op=mybir.AluOpType.add)
            nc.sync.dma_start(znq_s[rows, :], znq[:])

            # --- 3-way fp16 split into phi [128, 64(j), 54] ---
            phi = feat_p.tile([128, CG, K], F16, tag="phi")
            for bi in range(NBLK):
                nc.vector.memset(phi[:, :, 9 * bi + 8],
                                 1.0 if QPARTS[bi] == 0 else 0.0)

            def split_feat(src, featoff):
                # src [128, 256] viewed (g, j) -> slot cols featoff..featoff+4
                srcv = src[:].rearrange("p (g j) -> p j g", g=G)
                parts = [[bi for bi in range(NBLK) if QPARTS[bi] == w]
                         for w in range(3)]
                dsts = [[phi[:, :, 9 * bi + featoff:9 * bi + featoff + 4]
                         for bi in blks] for blks in parts]
                r1 = feat_p.tile([128, CG, G], F32, tag="spl_r1")
                r2 = feat_p.tile([128, CG, G], F32, tag="spl_r2")
                nc.vector.tensor_copy(dsts[0][0], srcv)
                for d in dsts[0][1:]:
                    nc.vector.tensor_copy(d, dsts[0][0])
                nc.vector.tensor_tensor(r1[:], srcv, dsts[0][0],
                                        op=mybir.AluOpType.subtract)
                nc.vector.tensor_copy(dsts[1][0], r1[:])
                for d in dsts[1][1:]:
                    nc.vector.tensor_copy(d, dsts[1][0])
                nc.vector.tensor_tensor(r2[:], r1[:], dsts[1][0],
                                        op=mybir.AluOpType.subtract)
                nc.vector.tensor_copy(dsts[2][0], r2[:])

            split_feat(av, 0)
            split_feat(iv, 4)

            # --- per chunk: transpose (SW-pipelined 1 ahead) + scores +
            #     argmax + gather ---
            rows = slice(128 * pb, 128 * (pb + 1))
            zq = work_p.tile([128, C], F32, tag="zq")
            zqv = zq[:].rearrange("p (j g) -> p j g", g=G)
            ft = ft_p.tile([K, CG * 128], F16, tag="ft")
            tp = tpsum_p.tile([K, 128], F16, tag="tp")
            nc.tensor.transpose(tp[:], phi[:, 0, :], ident[:])
            nc.scalar.copy(ft[:, 0:128], tp[:])
            for j in range(CG):
                if j + 1 < CG:
                    tp = tpsum_p.tile([K, 128], F16, tag="tp")
                    nc.tensor.transpose(tp[:], phi[:, j + 1, :], ident[:])
                    nc.scalar.copy(ft[:, 128 * (j + 1):128 * (j + 2)], tp[:])
                ps = psum_p.tile([128, NS], F32, tag="ps")
                lhsT = ft[:, 128 * j:128 * (j + 1)]
                nc.tensor.matmul(ps[:, 0:512], lhsT, frhs[:, 0:512],
                                 start=True, stop=True)
                nc.tensor.matmul(ps[:, 512:NS], lhsT, frhs[:, 512:NS],
                                 start=True, stop=True)
                m8 = idx_p.tile([128, 8], F32, tag="m8")
                nc.vector.max(m8[:], ps[:])
                cid = pb * CG + j
                i8 = idxcoll[:, 8 * cid:8 * cid + 8]
                nc.vector.max_index(i8, m8[:], ps[:])
                nc.gpsimd.indirect_dma_start(
                    out=zqv[:, j, :],
                    out_offset=None,
                    in_=prior[:],
                    in_offset=IndirectOffsetOnAxis(ap=idxcoll[:, 8 * cid:8 * cid + 1],
                                                   axis=0),
                )
                if dbg is not None and pb == 0 and j == 0:
                    nc.sync.dma_start(dbg[0:128, 0:NS], ps[:])
            zsb = work_p.tile([128, C], F32, tag="zsb")
            nc.scalar.copy(zsb[:].rearrange("p (g j) -> p j g", g=G),
                           zq[:].rearrange("p (j g) -> p j g", g=G))
            nc.sync.dma_start(zhat_s[rows, :], zsb[:])
            idxsm = idx_p.tile([128, CG], U32, tag="idxsm")
            nc.vector.tensor_copy(
                idxsm[:],
                idxcoll[:].rearrange("p (c e) -> p c e", e=8)[:, CG * pb:CG * (pb + 1), 0])
            nc.sync.dma_start(idx_s[rows, :], idxsm[:])

# ---------------------------------------------------------------------------
# host wrapper
# ---------------------------------------------------------------------------
_nc_cache = {}


def _get_nc(debug=None):
    key = ("k", tuple(debug) if debug else None)
    if key not in _nc_cache:
        _nc_cache[key] = build_kernel(debug)
    return _nc_cache[key]


def kernel(z, eps, prior_samples, _trace=False, _debug=None):
    z = np.ascontiguousarray(np.asarray(z, dtype=np.float32))
    eps = np.ascontiguousarray(np.asarray(eps, dtype=np.float32))
    prior = np.ascontiguousarray(np.asarray(prior_samples, dtype=np.float32))

    zflat = z.reshape(POS, C2)
    eflat = eps.reshape(POS, C)

    nc = _get_nc(_debug)
    in_maps = []
    for m in range(NCORES):
        sl = slice(POS_S * m, POS_S * (m + 1))
        in_maps.append({
            "zs": np.ascontiguousarray(zflat[sl]),
            "eps_s": np.ascontiguousarray(eflat[sl]),
            "prior": prior,
        })
    res = bass_utils.run_bass_kernel_spmd(
        nc, in_maps, core_ids=list(range(NCORES)), trace=_trace)

    zhat = np.concatenate([res.results[m]["zhat_s"] for m in range(NCORES)], axis=0)
    znq = np.concatenate([res.results[m]["znq_s"] for m in range(NCORES)], axis=0)
    idx = np.concatenate([res.results[m]["idx_s"] for m in range(NCORES)], axis=0)

    zhat = zhat.reshape(B, L, C)
    znq = znq.reshape(B, L, C)
    indices = idx.view(np.int32).reshape(B, L, CG)
    out = (zhat, znq, indices)
    if _trace or _debug:
        return out, res
    return out
